# revision 20
# baseline (speedup 1.0000x reference)
# Depthwise causal conv1d (B=8, T=4096, C=1024, K=4, dilation=1) on 8 TRN2
# NeuronCores.
#
# Math: y[b, t, c] = sum_{j=0..3} weight[c, 3-j] * x[b, t-j, c]   (x[t<0] = 0)
#
# Strategy:
#   - Shard batch: core b handles x[b] (one full (T, C) slice).
#   - Host transposes each shard to (C, T) so the time axis is contiguous in
#     DRAM and lands on the SBUF free dimension; channels land on partitions.
#   - On-chip: for each 128-channel block, one [128, T+3] SBUF tile (3-col
#     zero halo at the left edge).  The 4 taps are applied by the TensorEngine
#     as 4 accumulating matmuls with a per-block *diagonal* weight matrix
#     lhsT = diag(w[cblock, 3-j]) against time-shifted rhs slices; PSUM does
#     the 4-tap accumulation for free.  fp32r keeps the PE at 1 cycle/row.
#   - DVE/ACT alternate on PSUM->SBUF copies; HWDGE DMAs move 2MB rows.
#   - Host transposes results back and stacks to (B, T, C).

import numpy as np

B, T, C, K = 8, 4096, 1024, 4
N_CORES = 8
P = 128  # SBUF partitions
NSUB = 512  # matmul free-dim (one fp32 PSUM bank)
HALO = 4  # leading zero columns (causal left pad), shipped from host

_CACHE = {}


def _build_nc(t_len=T, n_ch=C, mode="f32r"):
    import concourse.mybir as mybir
    import concourse.tile as tile
    from concourse import bacc

    f32 = mybir.dt.float32
    if mode == "f32r":
        cdt = mybir.dt.float32r
    elif mode == "bf16":
        cdt = mybir.dt.bfloat16
    else:
        cdt = f32
    ncb = n_ch // P  # channel blocks
    nsub = t_len // NSUB  # time sub-blocks per channel block

    # Bacc (not raw Bass): its compile() pass legalizes multi-wait sync into
    # event-semaphore instructions (TRN2 allows 1 wait per instruction).
    nc = bacc.Bacc(None)
    # x is declared with the compute dtype; for f32r this is a bit-identical
    # view of f32, for bf16 the (SWDGE) DMA casts inline.
    x_dt = cdt if mode == "f32r" else f32
    x = nc.declare_dram_parameter("x", [n_ch, t_len + HALO], x_dt, isOutput=False)
    # w holds host-prebuilt diagonal blocks: slot (cb, j) at columns
    # [(cb*K+j)*128, ...) is diag(weight[cb*128 + p, K-1-j]).
    w = nc.declare_dram_parameter("w", [P, ncb * K * P], x_dt, isOutput=False)
    y = nc.declare_dram_parameter("y", [n_ch, t_len], f32, isOutput=True)

    with tile.TileContext(nc) as tc:
        with (
            tc.tile_pool(name="const", bufs=1) as cpool,
            tc.tile_pool(name="xin", bufs=2) as xpool,
            tc.tile_pool(name="yout", bufs=2) as ypool,
            tc.tile_pool(name="ps", bufs=8, space="PSUM") as pspool,
        ):
            wdiag = cpool.tile([P, ncb * K * P], cdt)
            wdma = nc.gpsimd if mode == "bf16" else nc.sync
            wdma.dma_start(out=wdiag[:, :], in_=w[:, :])

            for cb in range(ncb):
                # [P, HALO zero cols + t_len]: fp32r matmuls need even N, so
                # every matmul stays at N=512; the causal zero pad comes in
                # with the DMA (host ships x with HALO leading zero cols).
                xt = xpool.tile([P, t_len + HALO], cdt)
                xdma = nc.gpsimd if mode == "bf16" else nc.sync
                xdma.dma_start(out=xt[:, :], in_=x[cb * P : (cb + 1) * P, :])
                yt = ypool.tile([P, t_len], f32)
                for m in range(nsub):
                    ps = pspool.tile([P, NSUB], f32)
                    for j in range(K):
                        # y[:, t] += diag(w[:, K-1-j]) @ x[:, t - j]
                        s = (cb * K + j) * P
                        lhsT = wdiag[:, s : s + P]
                        off = NSUB * m + HALO - j
                        rhs = xt[:, off : off + NSUB]
                        nc.tensor.matmul(
                            ps[:, :], lhsT, rhs, start=(j == 0), stop=(j == K - 1)
                        )
                    dst = yt[:, NSUB * m : NSUB * (m + 1)]
                    if m % 2 == 0:
                        nc.vector.tensor_copy(dst, ps[:, :])
                    else:
                        nc.scalar.copy(dst, ps[:, :])
                nc.sync.dma_start(out=y[cb * P : (cb + 1) * P, :], in_=yt[:, :])
    return nc


def _get_nc():
    if "nc" not in _CACHE:
        nc = _build_nc()
        # Bacc.finalize() runs compile(): moves matmul waits to ldweights,
        # splits multi-wait sync into event-sem instructions, allocates regs.
        nc.finalize()
        _CACHE["nc"] = nc
    return _CACHE["nc"]


def _pack_weight(weight):
    # Diagonal blocks: wd[p, (cb*K + j)*P + m] = (p == m) * weight[cb*P + p, K-1-j]
    w = np.asarray(weight, dtype=np.float32)
    ncb = C // P
    wd = np.zeros((P, ncb * K, P), dtype=np.float32)
    idx = np.arange(P)
    for cb in range(ncb):
        for j in range(K):
            wd[idx, cb * K + j, idx] = w[cb * P + idx, K - 1 - j]
    return np.ascontiguousarray(wd.reshape(P, ncb * K * P))


LAST_RESULT = None


def kernel(x, weight):
    global LAST_RESULT
    from concourse.bass_utils import run_bass_kernel_spmd

    x = np.asarray(x, dtype=np.float32)
    w_sb = _pack_weight(weight)
    nc = _get_nc()

    in_maps = []
    for b in range(N_CORES):
        xt = np.zeros((C, T + HALO), dtype=np.float32)
        xt[:, HALO:] = x[b].T
        in_maps.append({"x": xt, "w": w_sb})
    res = run_bass_kernel_spmd(nc, in_maps, list(range(N_CORES)))
    LAST_RESULT = res

    y = np.empty((B, T, C), dtype=np.float32)
    for b in range(N_CORES):
        y[b] = res.results[b]["y"].T
    return y


# revision 22
# speedup vs baseline: 1.1347x; 1.1347x over previous
# Depthwise causal conv1d (B=8, T=4096, C=1024, K=4, dilation=1) on 8 TRN2
# NeuronCores.
#
# Math: y[b, t, c] = sum_{j=0..3} weight[c, 3-j] * x[b, t-j, c]   (x[t<0] = 0)
#
# Strategy:
#   - Shard batch: core b handles x[b] (one full (T, C) slice).
#   - Host transposes each shard to (C, T) so the time axis is contiguous in
#     DRAM and lands on the SBUF free dimension; channels land on partitions.
#   - On-chip: for each 128-channel block, one [128, T+3] SBUF tile (3-col
#     zero halo at the left edge).  The 4 taps are applied by the TensorEngine
#     as 4 accumulating matmuls with a per-block *diagonal* weight matrix
#     lhsT = diag(w[cblock, 3-j]) against time-shifted rhs slices; PSUM does
#     the 4-tap accumulation for free.  fp32r keeps the PE at 1 cycle/row.
#   - DVE/ACT alternate on PSUM->SBUF copies; HWDGE DMAs move 2MB rows.
#   - Host transposes results back and stacks to (B, T, C).

import numpy as np

B, T, C, K = 8, 4096, 1024, 4
N_CORES = 8
P = 128  # SBUF partitions
NSUB = 512  # matmul free-dim (one fp32 PSUM bank)
HALO = 4  # leading zero columns (causal left pad), shipped from host

_CACHE = {}


def _build_nc(t_len=T, n_ch=C, mode="f32r"):
    import concourse.mybir as mybir
    import concourse.tile as tile
    from concourse import bacc

    f32 = mybir.dt.float32
    if mode == "f32r":
        cdt = mybir.dt.float32r
    elif mode == "bf16":
        cdt = mybir.dt.bfloat16
    else:
        cdt = f32
    ncb = n_ch // P  # channel blocks
    nsub = t_len // NSUB  # time sub-blocks per channel block

    # Bacc (not raw Bass): its compile() pass legalizes multi-wait sync into
    # event-semaphore instructions (TRN2 allows 1 wait per instruction).
    nc = bacc.Bacc(None)
    # x is declared with the compute dtype; for f32r this is a bit-identical
    # view of f32, for bf16 the (SWDGE) DMA casts inline.
    x_dt = cdt if mode == "f32r" else f32
    x = nc.declare_dram_parameter("x", [n_ch, t_len + HALO], x_dt, isOutput=False)
    # w holds host-prebuilt diagonal blocks: slot (cb, j) at columns
    # [(cb*K+j)*128, ...) is diag(weight[cb*128 + p, K-1-j]).
    w = nc.declare_dram_parameter("w", [P, ncb * K * P], x_dt, isOutput=False)
    y = nc.declare_dram_parameter("y", [n_ch, t_len], f32, isOutput=True)

    with tile.TileContext(nc) as tc:
        with (
            tc.tile_pool(name="const", bufs=1) as cpool,
            tc.tile_pool(name="xin", bufs=3) as xpool,
            tc.tile_pool(name="yout", bufs=2) as ypool,
            tc.tile_pool(name="ps", bufs=8, space="PSUM") as pspool,
        ):
            wdiag = cpool.tile([P, ncb * K * P], cdt)
            wdma = nc.gpsimd if mode == "bf16" else nc.sync
            wdma.dma_start(out=wdiag[:, :], in_=w[:, :])

            for cb in range(ncb):
                # [P, HALO zero cols + t_len]: fp32r matmuls need even N, so
                # every matmul stays at N=512; the causal zero pad comes in
                # with the DMA (host ships x with HALO leading zero cols).
                xt = xpool.tile([P, t_len + HALO], cdt)
                xdma = nc.gpsimd if mode == "bf16" else nc.sync
                xdma.dma_start(out=xt[:, :], in_=x[cb * P : (cb + 1) * P, :])
                yt = ypool.tile([P, t_len], f32)
                for m in range(nsub):
                    ps = pspool.tile([P, NSUB], f32)
                    for j in range(K):
                        # y[:, t] += diag(w[:, K-1-j]) @ x[:, t - j]
                        s = (cb * K + j) * P
                        lhsT = wdiag[:, s : s + P]
                        off = NSUB * m + HALO - j
                        rhs = xt[:, off : off + NSUB]
                        nc.tensor.matmul(
                            ps[:, :], lhsT, rhs, start=(j == 0), stop=(j == K - 1)
                        )
                    dst = yt[:, NSUB * m : NSUB * (m + 1)]
                    if m % 2 == 0:
                        nc.vector.tensor_copy(dst, ps[:, :])
                    else:
                        nc.scalar.copy(dst, ps[:, :])
                # Stores go out on the ACT HWDGE ring (nc.scalar) so they
                # don't head-of-line-block the x loads on the SP ring.
                nc.scalar.dma_start(out=y[cb * P : (cb + 1) * P, :], in_=yt[:, :])
    return nc


def _get_nc():
    if "nc" not in _CACHE:
        nc = _build_nc()
        # Bacc.finalize() runs compile(): moves matmul waits to ldweights,
        # splits multi-wait sync into event-sem instructions, allocates regs.
        nc.finalize()
        _CACHE["nc"] = nc
    return _CACHE["nc"]


def _pack_weight(weight):
    # Diagonal blocks: wd[p, (cb*K + j)*P + m] = (p == m) * weight[cb*P + p, K-1-j]
    w = np.asarray(weight, dtype=np.float32)
    ncb = C // P
    wd = np.zeros((P, ncb * K, P), dtype=np.float32)
    idx = np.arange(P)
    for cb in range(ncb):
        for j in range(K):
            wd[idx, cb * K + j, idx] = w[cb * P + idx, K - 1 - j]
    return np.ascontiguousarray(wd.reshape(P, ncb * K * P))


LAST_RESULT = None


def kernel(x, weight):
    global LAST_RESULT
    from concourse.bass_utils import run_bass_kernel_spmd

    x = np.asarray(x, dtype=np.float32)
    w_sb = _pack_weight(weight)
    nc = _get_nc()

    in_maps = []
    for b in range(N_CORES):
        xt = np.zeros((C, T + HALO), dtype=np.float32)
        xt[:, HALO:] = x[b].T
        in_maps.append({"x": xt, "w": w_sb})
    res = run_bass_kernel_spmd(nc, in_maps, list(range(N_CORES)))
    LAST_RESULT = res

    y = np.empty((B, T, C), dtype=np.float32)
    for b in range(N_CORES):
        y[b] = res.results[b]["y"].T
    return y
